# revision 1
# baseline (speedup 1.0000x reference)
"""Multi-head attention (B=256, T=256, H=6, D=64, C=384) on 8 TRN2 NeuronCores.

Data-parallel over batch: each core owns 32 batch elements, all weights
replicated, no collectives. All matmuls fp32r (full-rate f32; self-loading
weights — bf16's separate LDWEIGHTS path crashes the PE on sub-128-row tile
transitions, so f32r everywhere like the proven baseline).

Per batch element, software-pipelined at head-PAIR granularity so the PE
never idles (any idle gap resets the tensor-engine p-state to half clock):

  stage      PE work                        emitted for   drained by
  qk_p       6 mm N=256 -> qk_ps (1 bank)   batch b+2     ACT copy -> f32r
  scores_p   4 mm N=256 (s0+s1 per side)    batch b+1     ACT exp -> p tiles
                                                          Pool masks/zerofill
  AV_p       4 mm N=256 accumulate          batch b/b+1   DVE recip + mults
             [v|ones]^T @ p                               (denoms = ones rows)
  v_s        3 mm N=384                     batch b+2     ACT/DVE copy -> v_aug
  proj_t     3 mm N=384                     batch b-1/b   DVE bias-add, DMA

PE emission order per iteration i (b == i), 3 pair-slots:
  slot0: scores_p0(b+1) qk_p0(b+2) AV_p1(b)
  slot1: qk_p1(b+2) scores_p1(b+1) proj_t1(b-1) AV_p2(b) v_s0(b+2)
  slot2: qk_p2(b+2) scores_p2(b+1) AV_p0(b+1) v_s1(b+2) proj_t0(b)

PSUM (8 banks): w x2, qk x2, o x2, v x1, out x1.
Engine budgets/batch: PE ~6.4us, ACT ~5.4 (exps, qk copies, v s0 copy),
DVE ~5.6 (recip, norm mults, biases, v s1 copy), Pool ~4.1 (masks).
"""

import numpy as np

import concourse.bacc as bacc
import concourse.mybir as mybir
import concourse.tile as tile
from concourse.bass_utils import run_bass_kernel_spmd

F32 = mybir.dt.float32
F32R = mybir.dt.float32r

N_CORES = 8
B, T, C = 256, 256, 384
H, D = 6, 64
B_LOC = B // N_CORES  # 32
SCALE = 1.0 / float(np.sqrt(np.float32(C)))


def build_nc(b_loc=B_LOC):
    nc = bacc.Bacc("TRN2", target_bir_lowering=False, debug=False)

    xt_d = nc.dram_tensor("xt", [b_loc, C, T], F32R, kind="ExternalInput")
    wqk_d = nc.dram_tensor("wqk", [3, 128, 768], F32R, kind="ExternalInput")
    wv_d = nc.dram_tensor("wv", [3, 128, 384], F32R, kind="ExternalInput")
    wp_d = nc.dram_tensor("wp", [3, 128, 384], F32R, kind="ExternalInput")
    bias_d = nc.dram_tensor("bias", [128, 384], F32, kind="ExternalInput")
    out_d = nc.dram_tensor("out", [b_loc, T, C], F32, kind="ExternalOutput")

    with tile.TileContext(nc) as tc:
        with (
            tc.tile_pool(name="const", bufs=1) as cpool,
            tc.tile_pool(name="xt", bufs=3) as xt_pool,
            tc.tile_pool(name="qksb", bufs=6) as qksb_pool,
            tc.tile_pool(name="p0", bufs=4) as p0_pool,
            tc.tile_pool(name="p1", bufs=4) as p1_pool,
            tc.tile_pool(name="osb", bufs=6) as osb_pool,
            tc.tile_pool(name="outsb", bufs=3) as outsb_pool,
            tc.tile_pool(name="rsb", bufs=2) as rsb_pool,
            tc.tile_pool(name="pw", bufs=1, space="PSUM") as pw,
            tc.tile_pool(name="pqk", bufs=2, space="PSUM") as pqk,
            tc.tile_pool(name="po", bufs=2, space="PSUM") as po,
            tc.tile_pool(name="pv", bufs=1, space="PSUM") as pv,
            tc.tile_pool(name="pout", bufs=1, space="PSUM") as pout,
        ):
            wqk = cpool.tile([128, 3, 768], F32R)
            wv = cpool.tile([128, 3, 384], F32R)
            wp = cpool.tile([128, 3, 384], F32R)
            bias = cpool.tile([128, 384], F32)
            # separate DGE queues so the startup loads run in parallel
            nc.sync.dma_start(wqk[:], wqk_d.ap().rearrange("k p m -> p k m"))
            nc.scalar.dma_start(wv[:], wv_d.ap().rearrange("k p m -> p k m"))
            nc.gpsimd.dma_start(wp[:], wp_d.ap().rearrange("k p m -> p k m"))
            nc.scalar.dma_start(bias[:], bias_d.ap())

            # v_aug ring: [v_h (64) | ones (64)] per head; ones written once.
            v_ring = []
            for r in range(4):
                v_aug = cpool.tile([128, 6, 128], F32R, name=f"v_aug{r}")
                nc.gpsimd.memset(v_aug[:, :, 64:128].bitcast(F32), 1.0)
                v_ring.append(v_aug)

            xt_t, qk_sb, p0_sb, p1_sb, o_sb = {}, {}, {}, {}, {}

            def e_dma_xt(b):
                xt = xt_pool.tile([128, 3, 256], F32R, tag="xt", name="xt_sb")
                nc.sync.dma_start(xt[:], xt_d.ap()[b].rearrange("(k p) t -> p k t", p=128))
                xt_t[b] = xt

            qk_ps_t = {}

            def e_qk_mm(b, p):
                qk_ps = pqk.tile([128, 512], F32, tag="qk", name="qk_ps")
                for qk in range(2):
                    m = (p * 2 + qk) * 128
                    for k in range(3):
                        nc.tensor.matmul(
                            qk_ps[:, qk * 256:(qk + 1) * 256],
                            wqk[:, k, m:m + 128],
                            xt_t[b][:, k, :],
                            start=(k == 0), stop=(k == 2),
                        )
                qk_ps_t[(b, p)] = qk_ps

            def e_qk_copy(b, p):
                # emitted after e_scores so the exps (which free the W bank
                # for the next pair) run ahead of this copy in the ACT queue
                sb = qksb_pool.tile([128, 512], F32R, tag="qksb", name="qk_sb_t")
                nc.scalar.activation(sb[:], qk_ps_t.pop((b, p)),
                                     mybir.ActivationFunctionType.Copy)
                qk_sb[(b, p)] = sb

            def e_scores(b, p):
                # transposed scores wT[s, t] = q_s . k_t per head side.
                # W layout [side, s, t]: each side's two K=64 matmuls stay in
                # ONE bank (side = bank). Sub-128-row weight loads at
                # different row offsets into the same PSUM bank back-to-back
                # crash the PE - sides must live in different banks (this is
                # what the baseline's SLOT permutation was for).
                w = pw.tile([128, 2, 2, 256], F32, tag="w", name="w_ps")
                qsb = qk_sb.pop((b, p))
                for side in range(2):
                    lo = side * 64
                    nc.tensor.matmul(w[:, side, 1, :], qsb[lo:lo + 64, 128:256],
                                     qsb[lo:lo + 64, 256:512], start=True, stop=True)
                    nc.tensor.matmul(w[:, side, 0, :], qsb[lo:lo + 64, 0:128],
                                     qsb[lo:lo + 64, 256:512], start=True, stop=True)
                pa = p0_pool.tile([128, 2, 256], F32R, tag="p0", name="p0_t")
                pb = p1_pool.tile([128, 2, 256], F32R, tag="p1", name="p1_t")
                nc.scalar.activation(pb[:, :, 128:256], w[:, :, 1, 128:256],
                                     mybir.ActivationFunctionType.Exp, scale=SCALE)
                nc.scalar.activation(pa[:], w[:, :, 0, :],
                                     mybir.ActivationFunctionType.Exp, scale=SCALE)
                # causal: diag masks keep t >= s; s1's t<128 block is all-masked
                nc.gpsimd.affine_select(
                    out=pb[:, :, 0:128], in_=pb[:, :, 128:256],
                    compare_op=mybir.AluOpType.is_ge, fill=0.0,
                    base=-1, pattern=[[0, 2], [0, 128]], channel_multiplier=0,
                )
                nc.gpsimd.affine_select(
                    out=pb[:, :, 128:256], in_=pb[:, :, 128:256],
                    compare_op=mybir.AluOpType.is_ge, fill=0.0,
                    base=0, pattern=[[0, 2], [1, 128]], channel_multiplier=-1,
                )
                nc.gpsimd.affine_select(
                    out=pa[:, :, 0:128], in_=pa[:, :, 0:128],
                    compare_op=mybir.AluOpType.is_ge, fill=0.0,
                    base=0, pattern=[[0, 2], [1, 128]], channel_multiplier=-1,
                )
                p0_sb[(b, p)] = pa
                p1_sb[(b, p)] = pb

            def e_v(b, s):
                v_ps = pv.tile([128, 384], F32, tag="v", name="v_ps")
                for k in range(3):
                    nc.tensor.matmul(
                        v_ps[:],
                        xt_t[b][:, k, s * 128:(s + 1) * 128],
                        wv[:, k, :],
                        start=(k == 0), stop=(k == 2),
                    )
                v_aug = v_ring[(2 * b + s) % 4]
                if s == 0:
                    nc.scalar.activation(
                        v_aug[:, :, 0:64],
                        v_ps[:].rearrange("p (h d) -> p h d", h=6),
                        mybir.ActivationFunctionType.Copy,
                    )
                else:
                    nc.vector.tensor_copy(
                        v_aug[:, :, 0:64],
                        v_ps[:].rearrange("p (h d) -> p h d", h=6),
                    )

            def e_av(b, p):
                o_ps = po.tile([128, 512], F32, tag="o", name="o_ps")
                pa = p0_sb.pop((b, p))
                pb = p1_sb.pop((b, p))
                va = v_ring[(2 * b) % 4]
                vb = v_ring[(2 * b + 1) % 4]
                for side in range(2):
                    h = 2 * p + side
                    base = side * 256
                    nc.tensor.matmul(o_ps[:, base:base + 256], va[:, h, :],
                                     pa[:, side, :], start=True, stop=False)
                    nc.tensor.matmul(o_ps[:, base:base + 256], vb[:, h, :],
                                     pb[:, side, :], start=False, stop=True)
                sb = osb_pool.tile([128, 256], F32R, tag="osb", name="o_sb_t")
                r = rsb_pool.tile([128, 512], F32, tag="r", name="r_sb")
                # custom DVE ops ignore partition offsets on HW: full range
                nc.vector.reciprocal_approx_fast(out=r[:], in_=o_ps[:])
                nc.vector.tensor_tensor(sb[0:64, :], o_ps[0:64, 0:256],
                                        r[64:128, 0:256], mybir.AluOpType.mult)
                nc.vector.tensor_tensor(sb[64:128, :], o_ps[0:64, 256:512],
                                        r[64:128, 256:512], mybir.AluOpType.mult)
                o_sb[(b, p)] = sb

            def e_proj(b, t):
                out_ps = pout.tile([128, 384], F32, tag="out", name="out_ps")
                for ch in range(3):
                    nc.tensor.matmul(
                        out_ps[:],
                        o_sb[(b, ch)][:, t * 128:(t + 1) * 128],
                        wp[:, ch, :],
                        start=(ch == 0), stop=(ch == 2),
                    )
                if t == 1:
                    for ch in range(3):
                        del o_sb[(b, ch)]
                out_sb = outsb_pool.tile([128, 384], F32, tag="outsb", name="out_sb_t")
                nc.vector.tensor_tensor(out_sb[:], out_ps[:], bias[:], mybir.AluOpType.add)
                nc.sync.dma_start(out_d.ap()[b, t * 128:(t + 1) * 128, :], out_sb[:])

            def g(b):
                return 0 <= b < b_loc

            e_dma_xt(0)
            e_dma_xt(1)
            for i in range(-2, b_loc + 1):
                # slot0
                if g(i + 2): e_qk_mm(i + 2, 0)
                if g(i + 1): e_scores(i + 1, 0)
                if g(i + 2): e_qk_copy(i + 2, 0)
                if g(i):     e_av(i, 1)
                # slot1
                if g(i + 2): e_qk_mm(i + 2, 1)
                if g(i + 1): e_scores(i + 1, 1)
                if g(i + 2): e_qk_copy(i + 2, 1)
                if g(i - 1): e_proj(i - 1, 1)
                if g(i):     e_av(i, 2)
                if g(i + 2): e_v(i + 2, 0)
                # slot2
                if g(i + 2): e_qk_mm(i + 2, 2)
                if g(i + 1): e_scores(i + 1, 2)
                if g(i + 2): e_qk_copy(i + 2, 2)
                if g(i + 1): e_av(i + 1, 0)
                if g(i + 2): e_v(i + 2, 1)
                if g(i):     e_proj(i, 0)
                if g(i + 4): e_dma_xt(i + 4)

    nc.compile()
    return nc


def _host_prep(x, wk, wq, wv, wproj, bproj):
    """Build the per-core input maps (host-side shard + repack)."""
    x = np.ascontiguousarray(x, dtype=np.float32)
    wk = np.asarray(wk, dtype=np.float32)
    wq = np.asarray(wq, dtype=np.float32)
    wv = np.asarray(wv, dtype=np.float32)
    wproj = np.asarray(wproj, dtype=np.float32)
    bproj = np.asarray(bproj, dtype=np.float32)

    # packed q/k weights: [pair, q/k, C, 128] -> [chunk(3), 128, 768]
    wqp = wq.reshape(3, 2, C, D)
    wkp = wk.reshape(3, 2, C, D)
    qk = np.empty((3, 2, C, 128), dtype=np.float32)
    qk[:, 0, :, 0:64] = wqp[:, 0]
    qk[:, 0, :, 64:128] = wqp[:, 1]
    qk[:, 1, :, 0:64] = wkp[:, 0]
    qk[:, 1, :, 64:128] = wkp[:, 1]
    wqk_h = np.ascontiguousarray(
        qk.transpose(2, 0, 1, 3).reshape(3, 128, 768))
    wv_h = np.ascontiguousarray(
        wv.transpose(1, 0, 2).reshape(C, H * D).reshape(3, 128, 384))
    wp_h = np.ascontiguousarray(wproj.reshape(3, 128, 384))
    bias_h = np.ascontiguousarray(
        np.broadcast_to(bproj.reshape(1, 384), (128, 384)), dtype=np.float32)

    in_maps = []
    for c in range(N_CORES):
        xs = x[c * B_LOC:(c + 1) * B_LOC]  # [B_LOC, T, C]
        xt = np.ascontiguousarray(xs.transpose(0, 2, 1))  # [B_LOC, C, T]
        in_maps.append({
            "xt": xt, "wqk": wqk_h, "wv": wv_h, "wp": wp_h, "bias": bias_h,
        })
    return in_maps


_NC_CACHE = {}


def run(inputs, trace=False, **kw):
    """Run on the 8 NeuronCores; returns (output, BassKernelResults)."""
    if "nc" not in _NC_CACHE:
        _NC_CACHE["nc"] = build_nc()
    nc = _NC_CACHE["nc"]
    in_maps = _host_prep(
        inputs["x"], inputs["wk"], inputs["wq"], inputs["wv"],
        inputs["wproj"], inputs["bproj"],
    )
    res = run_bass_kernel_spmd(nc, in_maps, core_ids=list(range(N_CORES)),
                               trace=trace, **kw)
    out = np.concatenate([res.results[c]["out"] for c in range(N_CORES)], axis=0)
    return out, res


def kernel(**inputs):
    inputs = {k: np.asarray(v, dtype=np.float32) for k, v in inputs.items()}
    out, _ = run(inputs, trace=False)
    return out



# revision 2
# speedup vs baseline: 1.0663x; 1.0663x over previous
"""Multi-head attention (B=256, T=256, H=6, D=64, C=384) on 8 TRN2 NeuronCores.

bf16 redesign of the fp32r baseline. Data-parallel over batch: each core owns
32 batch elements, weights replicated, no collectives.

Gains over fp32r baseline:
  - all matmuls bf16 (1 cyc/col at ANY N; fp32r needs N>=256, and its 191ns
    self-loading LDWEIGHTS hiccups; bf16 LDW is 95ns and fully hidden)
  - causal split: scores s1 computes only t'>=128 (N=128 at full rate);
    AV right half accumulates chunk b with N=128 matmuls; pb left block
    (all-masked) never computed, so no Pool zerofill
  - single merged exp per pair (768 cols) instead of 2 ACT calls
  - elementwise ops emit bf16 (halves SBUF traffic), output DMA'd as bf16

Per-batch stage structure and slot pipeline retained from the baseline
(3 head-pair slots; stages offset by batch so the PE never idles):

  stage        PE work                         drained by
  qk_p         6 mm N=256 -> qk_ps (1 bank)    ACT/DVE copy -> bf16
  scores_p     s0 N=256 + s1 N=128 per side,   ACT exp (1 call, 768 cols)
               sides row-paired (K=64)         Pool triangle masks x2
  AV_p         6 mm N=128 accumulate           DVE recip + 2 TT mult -> bf16
               [v|ones]^T @ p
  v_s          3 mm N=384 (bf16)               ACT copy s0 / DVE copy s1
  proj_t       3 mm N=384 (bf16)               ACT bias-add -> bf16, DMA

PSUM (8 banks): w 2 (1/side), qk x2, o x2, v x1, out x1.
"""

import numpy as np
import ml_dtypes

import concourse.bacc as bacc
import concourse.mybir as mybir
import concourse.tile as tile
from concourse.bass_utils import run_bass_kernel_spmd

F32 = mybir.dt.float32
BF16 = mybir.dt.bfloat16
AF = mybir.ActivationFunctionType

N_CORES = 8
B, T, C = 256, 256, 384
H, D = 6, 64
B_LOC = B // N_CORES  # 32
SCALE = 1.0 / float(np.sqrt(np.float32(C)))


def build_nc(b_loc=B_LOC):
    nc = bacc.Bacc("TRN2", target_bir_lowering=False, debug=False)

    xt_d = nc.dram_tensor("xt", [b_loc, C, T], BF16, kind="ExternalInput")
    wqk_d = nc.dram_tensor("wqk", [3, 128, 768], BF16, kind="ExternalInput")
    wv_d = nc.dram_tensor("wv", [3, 128, 384], BF16, kind="ExternalInput")
    wp_d = nc.dram_tensor("wp", [3, 128, 384], BF16, kind="ExternalInput")
    bias_d = nc.dram_tensor("bias", [128, 384], F32, kind="ExternalInput")
    out_d = nc.dram_tensor("out", [b_loc, T, C], BF16, kind="ExternalOutput")

    with tile.TileContext(nc) as tc:
        with (
            tc.tile_pool(name="const", bufs=1) as cpool,
            tc.tile_pool(name="xt", bufs=3) as xt_pool,
            tc.tile_pool(name="qksb", bufs=6) as qksb_pool,
            tc.tile_pool(name="p", bufs=4) as p_pool,
            tc.tile_pool(name="osb", bufs=6) as osb_pool,
            tc.tile_pool(name="outsb", bufs=3) as outsb_pool,
            tc.tile_pool(name="rsb", bufs=2) as rsb_pool,
            tc.tile_pool(name="pw", bufs=1, space="PSUM") as pw,
            tc.tile_pool(name="pqk", bufs=2, space="PSUM") as pqk,
            tc.tile_pool(name="po", bufs=2, space="PSUM") as po,
            tc.tile_pool(name="pv", bufs=1, space="PSUM") as pv,
            tc.tile_pool(name="pout", bufs=1, space="PSUM") as pout,
        ):
            wqk = cpool.tile([128, 3, 768], BF16)
            wv = cpool.tile([128, 3, 384], BF16)
            wp = cpool.tile([128, 3, 384], BF16)
            bias = cpool.tile([128, 384], F32)
            nc.sync.dma_start(wqk[:], wqk_d.ap().rearrange("k p m -> p k m"))
            nc.scalar.dma_start(wv[:], wv_d.ap().rearrange("k p m -> p k m"))
            nc.gpsimd.dma_start(wp[:], wp_d.ap().rearrange("k p m -> p k m"))
            nc.scalar.dma_start(bias[:], bias_d.ap())

            # v_aug ring: [v_h (64) | ones (64)] per head; ones written once.
            v_ring = []
            for r in range(4):
                v_aug = cpool.tile([128, 6, 128], BF16, name=f"v_aug{r}")
                nc.gpsimd.memset(v_aug[:, :, 64:128], 1.0)
                v_ring.append(v_aug)

            xt_t, qk_sb, p_sb, o_sb = {}, {}, {}, {}

            def e_dma_xt(b):
                xt = xt_pool.tile([128, 3, 256], BF16, tag="xt", name="xt_sb")
                nc.sync.dma_start(xt[:], xt_d.ap()[b].rearrange("(k p) t -> p k t", p=128))
                xt_t[b] = xt

            qk_ps_t = {}

            def e_qk_mm(b, p):
                qk_ps = pqk.tile([128, 512], F32, tag="qk", name="qk_ps")
                for qk in range(2):
                    m = (p * 2 + qk) * 128
                    for k in range(3):
                        nc.tensor.matmul(
                            qk_ps[:, qk * 256:(qk + 1) * 256],
                            wqk[:, k, m:m + 128],
                            xt_t[b][:, k, :],
                            start=(k == 0), stop=(k == 2),
                        )
                qk_ps_t[(b, p)] = qk_ps

            def e_qk_copy(b, p):
                # PSUM f32 -> SBUF bf16. Alternate engines to balance load:
                # pair 1 on DVE, pairs 0/2 on ACT.
                sb = qksb_pool.tile([128, 512], BF16, tag="qksb", name="qk_sb_t")
                src = qk_ps_t.pop((b, p))
                if p == 1:
                    nc.vector.tensor_copy(sb[:], src[:])
                else:
                    nc.scalar.activation(sb[:], src[:], AF.Copy)
                qk_sb[(b, p)] = sb

            def e_scores(b, p):
                # w[s, t'] = q_s . k_t' per side; sides in separate banks so
                # the K=64 row-paired matmuls (rows 0:64 / 64:128) overlap.
                # Layout per side: cols 0:256 = s-chunk0 (t' 0:256),
                # cols 256:384 = s-chunk1 right half (t' 128:256).
                w = pw.tile([128, 2, 512], F32, tag="w", name="w_ps")
                qsb = qk_sb.pop((b, p))
                for sc, (qlo, qhi, klo, khi, wlo, whi) in enumerate(
                    ((0, 128, 256, 512, 0, 256), (128, 256, 384, 512, 256, 384))
                ):
                    for side in range(2):
                        lo = side * 64
                        nc.tensor.matmul(
                            w[:, side, wlo:whi],
                            qsb[lo:lo + 64, qlo:qhi],
                            qsb[lo:lo + 64, klo:khi],
                            start=True, stop=True,
                        )
                pt = p_pool.tile([128, 2, 384], BF16, tag="p", name="p_t")
                nc.scalar.activation(pt[:], w[:, :, 0:384], AF.Exp, scale=SCALE)
                # causal triangles (keep t' >= s) on the two diagonal blocks
                nc.gpsimd.affine_select(
                    out=pt[:, :, 0:128], in_=pt[:, :, 0:128],
                    compare_op=mybir.AluOpType.is_ge, fill=0.0,
                    base=0, pattern=[[0, 2], [1, 128]], channel_multiplier=-1,
                )
                nc.gpsimd.affine_select(
                    out=pt[:, :, 256:384], in_=pt[:, :, 256:384],
                    compare_op=mybir.AluOpType.is_ge, fill=0.0,
                    base=0, pattern=[[0, 2], [1, 128]], channel_multiplier=-1,
                )
                p_sb[(b, p)] = pt

            def e_v(b, s):
                v_ps = pv.tile([128, 384], F32, tag="v", name="v_ps")
                for k in range(3):
                    nc.tensor.matmul(
                        v_ps[:],
                        xt_t[b][:, k, s * 128:(s + 1) * 128],
                        wv[:, k, :],
                        start=(k == 0), stop=(k == 2),
                    )
                v_aug = v_ring[(2 * b + s) % 4]
                if s == 0:
                    nc.scalar.activation(
                        v_aug[:, :, 0:64],
                        v_ps[:].rearrange("p (h d) -> p h d", h=6),
                        AF.Copy,
                    )
                else:
                    nc.vector.tensor_copy(
                        v_aug[:, :, 0:64],
                        v_ps[:].rearrange("p (h d) -> p h d", h=6),
                    )

            def e_av(b, p):
                o_ps = po.tile([128, 512], F32, tag="o", name="o_ps")
                pt = p_sb.pop((b, p))
                va = v_ring[(2 * b) % 4]
                vb = v_ring[(2 * b + 1) % 4]
                for side in range(2):
                    h = 2 * p + side
                    base = side * 256
                    nc.tensor.matmul(o_ps[:, base:base + 128], va[:, h, :],
                                     pt[:, side, 0:128], start=True, stop=True)
                    nc.tensor.matmul(o_ps[:, base + 128:base + 256], va[:, h, :],
                                     pt[:, side, 128:256], start=True, stop=False)
                    nc.tensor.matmul(o_ps[:, base + 128:base + 256], vb[:, h, :],
                                     pt[:, side, 256:384], start=False, stop=True)
                sb = osb_pool.tile([128, 256], BF16, tag="osb", name="o_sb_t")
                r = rsb_pool.tile([128, 512], F32, tag="r", name="r_sb")
                # custom DVE ops ignore partition offsets on HW: full range
                nc.vector.reciprocal_approx_fast(out=r[:], in_=o_ps[:])
                nc.vector.tensor_tensor(sb[0:64, :], o_ps[0:64, 0:256],
                                        r[64:128, 0:256], mybir.AluOpType.mult)
                nc.vector.tensor_tensor(sb[64:128, :], o_ps[0:64, 256:512],
                                        r[64:128, 256:512], mybir.AluOpType.mult)
                o_sb[(b, p)] = sb

            def e_proj(b, t):
                out_ps = pout.tile([128, 384], F32, tag="out", name="out_ps")
                for ch in range(3):
                    nc.tensor.matmul(
                        out_ps[:],
                        o_sb[(b, ch)][:, t * 128:(t + 1) * 128],
                        wp[:, ch, :],
                        start=(ch == 0), stop=(ch == 2),
                    )
                if t == 1:
                    for ch in range(3):
                        del o_sb[(b, ch)]
                out_sb = outsb_pool.tile([128, 384], BF16, tag="outsb", name="out_sb_t")
                nc.vector.tensor_tensor(out_sb[:], out_ps[:], bias[:], mybir.AluOpType.add)
                nc.sync.dma_start(out_d.ap()[b, t * 128:(t + 1) * 128, :], out_sb[:])

            def g(b):
                return 0 <= b < b_loc

            e_dma_xt(0)
            e_dma_xt(1)
            for i in range(-2, b_loc + 1):
                # slot0
                if g(i + 2): e_qk_mm(i + 2, 0)
                if g(i + 1): e_scores(i + 1, 0)
                if g(i + 2): e_qk_copy(i + 2, 0)
                if g(i):     e_av(i, 1)
                # slot1
                if g(i + 2): e_qk_mm(i + 2, 1)
                if g(i + 1): e_scores(i + 1, 1)
                if g(i + 2): e_qk_copy(i + 2, 1)
                if g(i - 1): e_proj(i - 1, 1)
                if g(i):     e_av(i, 2)
                if g(i + 2): e_v(i + 2, 0)
                # slot2
                if g(i + 2): e_qk_mm(i + 2, 2)
                if g(i + 1): e_scores(i + 1, 2)
                if g(i + 2): e_qk_copy(i + 2, 2)
                if g(i + 1): e_av(i + 1, 0)
                if g(i + 2): e_v(i + 2, 1)
                if g(i):     e_proj(i, 0)
                if g(i + 4): e_dma_xt(i + 4)

    nc.compile()
    return nc


def _host_prep(x, wk, wq, wv, wproj, bproj):
    """Build the per-core input maps (host-side shard + repack + bf16 cast)."""
    x = np.ascontiguousarray(x, dtype=np.float32)
    wk = np.asarray(wk, dtype=np.float32)
    wq = np.asarray(wq, dtype=np.float32)
    wv = np.asarray(wv, dtype=np.float32)
    wproj = np.asarray(wproj, dtype=np.float32)
    bproj = np.asarray(bproj, dtype=np.float32)

    # packed q/k weights: [pair, q/k, C, 128] -> [chunk(3), 128, 768]
    wqp = wq.reshape(3, 2, C, D)
    wkp = wk.reshape(3, 2, C, D)
    qk = np.empty((3, 2, C, 128), dtype=np.float32)
    qk[:, 0, :, 0:64] = wqp[:, 0]
    qk[:, 0, :, 64:128] = wqp[:, 1]
    qk[:, 1, :, 0:64] = wkp[:, 0]
    qk[:, 1, :, 64:128] = wkp[:, 1]
    wqk_h = np.ascontiguousarray(
        qk.transpose(2, 0, 1, 3).reshape(3, 128, 768)).astype(ml_dtypes.bfloat16)
    wv_h = np.ascontiguousarray(
        wv.transpose(1, 0, 2).reshape(C, H * D).reshape(3, 128, 384)).astype(ml_dtypes.bfloat16)
    wp_h = np.ascontiguousarray(wproj.reshape(3, 128, 384)).astype(ml_dtypes.bfloat16)
    bias_h = np.ascontiguousarray(
        np.broadcast_to(bproj.reshape(1, 384), (128, 384)), dtype=np.float32)

    in_maps = []
    for c in range(N_CORES):
        xs = x[c * B_LOC:(c + 1) * B_LOC]  # [B_LOC, T, C]
        xt = np.ascontiguousarray(xs.transpose(0, 2, 1)).astype(ml_dtypes.bfloat16)
        in_maps.append({
            "xt": xt, "wqk": wqk_h, "wv": wv_h, "wp": wp_h, "bias": bias_h,
        })
    return in_maps


_NC_CACHE = {}


def run(inputs, trace=False, **kw):
    """Run on the 8 NeuronCores; returns (output, BassKernelResults)."""
    if "nc" not in _NC_CACHE:
        _NC_CACHE["nc"] = build_nc()
    nc = _NC_CACHE["nc"]
    in_maps = _host_prep(
        inputs["x"], inputs["wk"], inputs["wq"], inputs["wv"],
        inputs["wproj"], inputs["bproj"],
    )
    res = run_bass_kernel_spmd(nc, in_maps, core_ids=list(range(N_CORES)),
                               trace=trace, **kw)
    out = np.concatenate(
        [res.results[c]["out"].astype(np.float32) for c in range(N_CORES)], axis=0)
    return out, res


def kernel(**inputs):
    inputs = {k: np.asarray(v, dtype=np.float32) for k, v in inputs.items()}
    out, _ = run(inputs, trace=False)
    return out


# revision 3
# speedup vs baseline: 1.1071x; 1.0383x over previous
"""Multi-head attention (B=256, T=256, H=6, D=64, C=384) on 8 TRN2 NeuronCores.

bf16 redesign of the fp32r baseline. Data-parallel over batch: each core owns
32 batch elements, weights replicated, no collectives.

Gains over fp32r baseline:
  - all matmuls bf16 (1 cyc/col at ANY N; fp32r needs N>=256, and its 191ns
    self-loading LDWEIGHTS hiccups; bf16 LDW is 95ns and fully hidden)
  - causal split: scores s1 computes only t'>=128 (N=128 at full rate);
    AV right half accumulates chunk b with N=128 matmuls; pb left block
    (all-masked) never computed, so no Pool zerofill
  - single merged exp per pair (768 cols) instead of 2 ACT calls
  - elementwise ops emit bf16 (halves SBUF traffic), output DMA'd as bf16

Per-batch stage structure and slot pipeline retained from the baseline
(3 head-pair slots; stages offset by batch so the PE never idles):

  stage        PE work                         drained by
  qk_p         6 mm N=256 -> qk_ps (1 bank)    ACT/DVE copy -> bf16
  scores_p     s0 N=256 + s1 N=128 per side,   ACT exp (1 call, 768 cols)
               sides row-paired (K=64)         Pool triangle masks x2
  AV_p         6 mm N=128 accumulate           DVE recip + 2 TT mult -> bf16
               [v|ones]^T @ p
  v_s          3 mm N=384 (bf16)               ACT copy s0 / DVE copy s1
  proj_t       3 mm N=384 (bf16)               ACT bias-add -> bf16, DMA

PSUM (8 banks): w 2 (1/side), qk x2, o x2, v x1, out x1.
"""

import numpy as np
import ml_dtypes

import concourse.bacc as bacc
import concourse.mybir as mybir
import concourse.tile as tile
from concourse.bass_utils import run_bass_kernel_spmd

F32 = mybir.dt.float32
BF16 = mybir.dt.bfloat16
AF = mybir.ActivationFunctionType

N_CORES = 8
B, T, C = 256, 256, 384
H, D = 6, 64
B_LOC = B // N_CORES  # 32
SCALE = 1.0 / float(np.sqrt(np.float32(C)))


def build_nc(b_loc=B_LOC):
    nc = bacc.Bacc("TRN2", target_bir_lowering=False, debug=False)

    xt_d = nc.dram_tensor("xt", [b_loc, C, T], BF16, kind="ExternalInput")
    wqk_d = nc.dram_tensor("wqk", [3, 128, 768], BF16, kind="ExternalInput")
    wv_d = nc.dram_tensor("wv", [3, 128, 384], BF16, kind="ExternalInput")
    wp_d = nc.dram_tensor("wp", [3, 128, 384], BF16, kind="ExternalInput")
    bias_d = nc.dram_tensor("bias", [128, 384], F32, kind="ExternalInput")
    out_d = nc.dram_tensor("out", [b_loc, T, C], BF16, kind="ExternalOutput")

    with tile.TileContext(nc) as tc:
        with (
            tc.tile_pool(name="const", bufs=1) as cpool,
            tc.tile_pool(name="xt", bufs=3) as xt_pool,
            tc.tile_pool(name="qksb", bufs=6) as qksb_pool,
            tc.tile_pool(name="p", bufs=4) as p_pool,
            tc.tile_pool(name="osb", bufs=6) as osb_pool,
            tc.tile_pool(name="outsb", bufs=3) as outsb_pool,
            tc.tile_pool(name="rsb", bufs=2) as rsb_pool,
            tc.tile_pool(name="pw", bufs=1, space="PSUM") as pw,
            tc.tile_pool(name="pqk", bufs=2, space="PSUM") as pqk,
            tc.tile_pool(name="po", bufs=2, space="PSUM") as po,
            tc.tile_pool(name="pv", bufs=1, space="PSUM") as pv,
            tc.tile_pool(name="pout", bufs=1, space="PSUM") as pout,
        ):
            wqk = cpool.tile([128, 3, 768], BF16)
            wv = cpool.tile([128, 3, 384], BF16)
            wp = cpool.tile([128, 3, 384], BF16)
            bias = cpool.tile([128, 384], F32)
            nc.sync.dma_start(wqk[:], wqk_d.ap().rearrange("k p m -> p k m"))
            nc.scalar.dma_start(wv[:], wv_d.ap().rearrange("k p m -> p k m"))
            nc.gpsimd.dma_start(wp[:], wp_d.ap().rearrange("k p m -> p k m"))
            nc.scalar.dma_start(bias[:], bias_d.ap())

            # v_aug ring: [v_h (64) | ones (64)] per head; ones written once.
            v_ring = []
            for r in range(4):
                v_aug = cpool.tile([128, 6, 128], BF16, name=f"v_aug{r}")
                nc.gpsimd.memset(v_aug[:, :, 64:128], 1.0)
                v_ring.append(v_aug)

            xt_t, qk_sb, p_sb, o_sb = {}, {}, {}, {}

            def e_dma_xt(b):
                xt = xt_pool.tile([128, 3, 256], BF16, tag="xt", name="xt_sb")
                nc.sync.dma_start(xt[:], xt_d.ap()[b].rearrange("(k p) t -> p k t", p=128))
                xt_t[b] = xt

            qk_ps_t = {}

            def e_qk_mm(b, p):
                qk_ps = pqk.tile([128, 512], F32, tag="qk", name="qk_ps")
                for qk in range(2):
                    m = (p * 2 + qk) * 128
                    for k in range(3):
                        nc.tensor.matmul(
                            qk_ps[:, qk * 256:(qk + 1) * 256],
                            wqk[:, k, m:m + 128],
                            xt_t[b][:, k, :],
                            start=(k == 0), stop=(k == 2),
                        )
                qk_ps_t[(b, p)] = qk_ps

            def e_qk_copy(b, p):
                # PSUM f32 -> SBUF bf16. Alternate engines to balance load:
                # pair 1 on DVE, pairs 0/2 on ACT.
                sb = qksb_pool.tile([128, 512], BF16, tag="qksb", name="qk_sb_t")
                src = qk_ps_t.pop((b, p))
                nc.scalar.activation(sb[:], src[:], AF.Copy)
                qk_sb[(b, p)] = sb

            def e_scores(b, p):
                # w[s, t'] = q_s . k_t' per side; sides in separate banks so
                # the K=64 row-paired matmuls (rows 0:64 / 64:128) overlap.
                # Layout per side: cols 0:256 = s-chunk0 (t' 0:256),
                # cols 256:384 = s-chunk1 right half (t' 128:256).
                w = pw.tile([128, 2, 512], F32, tag="w", name="w_ps")
                qsb = qk_sb.pop((b, p))
                # s1 (short, N=128) first: its pair drains faster, shrinking
                # the window the scheduler fills with foreign full-row mms.
                for sc, (qlo, qhi, klo, khi, wlo, whi) in enumerate(
                    ((128, 256, 384, 512, 256, 384), (0, 128, 256, 512, 0, 256))
                ):
                    for side in range(2):
                        lo = side * 64
                        nc.tensor.matmul(
                            w[:, side, wlo:whi],
                            qsb[lo:lo + 64, qlo:qhi],
                            qsb[lo:lo + 64, klo:khi],
                            start=True, stop=True,
                        )
                pt = p_pool.tile([128, 2, 384], BF16, tag="p", name="p_t")
                nc.scalar.activation(pt[:], w[:, :, 0:384], AF.Exp, scale=SCALE)
                # causal triangles (keep t' >= s) on the two diagonal blocks
                nc.gpsimd.affine_select(
                    out=pt[:, :, 0:128], in_=pt[:, :, 0:128],
                    compare_op=mybir.AluOpType.is_ge, fill=0.0,
                    base=0, pattern=[[0, 2], [1, 128]], channel_multiplier=-1,
                )
                nc.gpsimd.affine_select(
                    out=pt[:, :, 256:384], in_=pt[:, :, 256:384],
                    compare_op=mybir.AluOpType.is_ge, fill=0.0,
                    base=0, pattern=[[0, 2], [1, 128]], channel_multiplier=-1,
                )
                p_sb[(b, p)] = pt

            def e_v(b, s):
                v_ps = pv.tile([128, 384], F32, tag="v", name="v_ps")
                for k in range(3):
                    nc.tensor.matmul(
                        v_ps[:],
                        xt_t[b][:, k, s * 128:(s + 1) * 128],
                        wv[:, k, :],
                        start=(k == 0), stop=(k == 2),
                    )
                v_aug = v_ring[(2 * b + s) % 4]
                if s == 0:
                    nc.scalar.activation(
                        v_aug[:, :, 0:64],
                        v_ps[:].rearrange("p (h d) -> p h d", h=6),
                        AF.Copy,
                    )
                else:
                    nc.vector.tensor_copy(
                        v_aug[:, :, 0:64],
                        v_ps[:].rearrange("p (h d) -> p h d", h=6),
                    )

            def e_av(b, p):
                o_ps = po.tile([128, 512], F32, tag="o", name="o_ps")
                pt = p_sb.pop((b, p))
                va = v_ring[(2 * b) % 4]
                vb = v_ring[(2 * b + 1) % 4]
                for side in range(2):
                    h = 2 * p + side
                    base = side * 256
                    # single N=256 va matmul; left half's group never sees a
                    # stop (no-op on HW), checker bypassed
                    nc.tensor.matmul(o_ps[:, base:base + 256], va[:, h, :],
                                     pt[:, side, 0:256], start=True, stop=False,
                                     skip_group_check=True)
                    nc.tensor.matmul(o_ps[:, base + 128:base + 256], vb[:, h, :],
                                     pt[:, side, 256:384], start=False, stop=True,
                                     skip_group_check=True)
                sb = osb_pool.tile([128, 256], BF16, tag="osb", name="o_sb_t")
                r = rsb_pool.tile([128, 512], F32, tag="r", name="r_sb")
                # custom DVE ops ignore partition offsets on HW: full range
                nc.vector.reciprocal_approx_fast(out=r[:], in_=o_ps[:])
                nc.vector.tensor_tensor(sb[0:64, :], o_ps[0:64, 0:256],
                                        r[64:128, 0:256], mybir.AluOpType.mult)
                nc.vector.tensor_tensor(sb[64:128, :], o_ps[0:64, 256:512],
                                        r[64:128, 256:512], mybir.AluOpType.mult)
                o_sb[(b, p)] = sb

            def e_proj(b, t):
                out_ps = pout.tile([128, 384], F32, tag="out", name="out_ps")
                for ch in range(3):
                    nc.tensor.matmul(
                        out_ps[:],
                        o_sb[(b, ch)][:, t * 128:(t + 1) * 128],
                        wp[:, ch, :],
                        start=(ch == 0), stop=(ch == 2),
                    )
                if t == 1:
                    for ch in range(3):
                        del o_sb[(b, ch)]
                out_sb = outsb_pool.tile([128, 384], BF16, tag="outsb", name="out_sb_t")
                nc.vector.tensor_tensor(out_sb[:], out_ps[:], bias[:], mybir.AluOpType.add)
                nc.sync.dma_start(out_d.ap()[b, t * 128:(t + 1) * 128, :], out_sb[:])

            def g(b):
                return 0 <= b < b_loc

            e_dma_xt(0)
            e_dma_xt(1)
            for i in range(-2, b_loc + 1):
                # slot0
                if g(i + 2): e_qk_mm(i + 2, 0)
                if g(i + 1): e_scores(i + 1, 0)
                if g(i + 2): e_qk_copy(i + 2, 0)
                if g(i):     e_av(i, 1)
                # slot1
                if g(i + 2): e_qk_mm(i + 2, 1)
                if g(i + 1): e_scores(i + 1, 1)
                if g(i + 2): e_qk_copy(i + 2, 1)
                if g(i - 1): e_proj(i - 1, 1)
                if g(i):     e_av(i, 2)
                if g(i + 2): e_v(i + 2, 0)
                # slot2
                if g(i + 2): e_qk_mm(i + 2, 2)
                if g(i + 1): e_scores(i + 1, 2)
                if g(i + 2): e_qk_copy(i + 2, 2)
                if g(i + 1): e_av(i + 1, 0)
                if g(i + 2): e_v(i + 2, 1)
                if g(i):     e_proj(i, 0)
                if g(i + 4): e_dma_xt(i + 4)

    nc.compile()
    return nc


def _host_prep(x, wk, wq, wv, wproj, bproj):
    """Build the per-core input maps (host-side shard + repack + bf16 cast)."""
    x = np.ascontiguousarray(x, dtype=np.float32)
    wk = np.asarray(wk, dtype=np.float32)
    wq = np.asarray(wq, dtype=np.float32)
    wv = np.asarray(wv, dtype=np.float32)
    wproj = np.asarray(wproj, dtype=np.float32)
    bproj = np.asarray(bproj, dtype=np.float32)

    # packed q/k weights: [pair, q/k, C, 128] -> [chunk(3), 128, 768]
    wqp = wq.reshape(3, 2, C, D)
    wkp = wk.reshape(3, 2, C, D)
    qk = np.empty((3, 2, C, 128), dtype=np.float32)
    qk[:, 0, :, 0:64] = wqp[:, 0]
    qk[:, 0, :, 64:128] = wqp[:, 1]
    qk[:, 1, :, 0:64] = wkp[:, 0]
    qk[:, 1, :, 64:128] = wkp[:, 1]
    wqk_h = np.ascontiguousarray(
        qk.transpose(2, 0, 1, 3).reshape(3, 128, 768)).astype(ml_dtypes.bfloat16)
    wv_h = np.ascontiguousarray(
        wv.transpose(1, 0, 2).reshape(C, H * D).reshape(3, 128, 384)).astype(ml_dtypes.bfloat16)
    wp_h = np.ascontiguousarray(wproj.reshape(3, 128, 384)).astype(ml_dtypes.bfloat16)
    bias_h = np.ascontiguousarray(
        np.broadcast_to(bproj.reshape(1, 384), (128, 384)), dtype=np.float32)

    in_maps = []
    for c in range(N_CORES):
        xs = x[c * B_LOC:(c + 1) * B_LOC]  # [B_LOC, T, C]
        xt = np.ascontiguousarray(xs.transpose(0, 2, 1)).astype(ml_dtypes.bfloat16)
        in_maps.append({
            "xt": xt, "wqk": wqk_h, "wv": wv_h, "wp": wp_h, "bias": bias_h,
        })
    return in_maps


_NC_CACHE = {}


def run(inputs, trace=False, **kw):
    """Run on the 8 NeuronCores; returns (output, BassKernelResults)."""
    if "nc" not in _NC_CACHE:
        _NC_CACHE["nc"] = build_nc()
    nc = _NC_CACHE["nc"]
    in_maps = _host_prep(
        inputs["x"], inputs["wk"], inputs["wq"], inputs["wv"],
        inputs["wproj"], inputs["bproj"],
    )
    res = run_bass_kernel_spmd(nc, in_maps, core_ids=list(range(N_CORES)),
                               trace=trace, **kw)
    out = np.concatenate(
        [res.results[c]["out"].astype(np.float32) for c in range(N_CORES)], axis=0)
    return out, res


def kernel(**inputs):
    inputs = {k: np.asarray(v, dtype=np.float32) for k, v in inputs.items()}
    out, _ = run(inputs, trace=False)
    return out
